# revision 29
# baseline (speedup 1.0000x reference)
"""Trainium2 Bass kernel for ClassicalSelfAttention.

  out = softmax((X @ R) @ (X @ E).T / sqrt(D)) @ X,  X: (8192, 1024) fp32

Key identity: scores = (X R)(X E)^T = X (R E^T) X^T.  Each core computes
W = R @ E^T redundantly (27us of PE), projects its own query slice
(T^T with lhsT=W, rhs=X^T_own), and then the "keys" are just X^T itself
— which every core already holds in DRAM.  No collectives at all: the
attention loop streams X^T / X blocks straight from HBM.

Sequence-parallel over 8 NeuronCores: core i owns queries
[i*1024, (i+1)*1024).  Attention runs over 16 key blocks of 512 in ring
order starting at the core's own two blocks, with a standard online
softmax (per-query running max on DVE, exp on ACT, PV accumulated in
per-half PSUM banks then merged into SBUF by DVE).  P^T for the PV
matmul is produced by PE transposes against a bf16 identity,
interleaved between the previous pending's PV matmuls so the
LDW-bound transposes hide under 213ns PV streams.

Precision: the softmax is extremely peaked (logit std ~1200 scaled;
argmax gaps down to ~3), so logits need >=11-bit operand mantissas —
bf16 flips argmaxes and fails.  QK runs in fp16 x fp16 (exact MACs,
one-pass LDWEIGHTS); W and T^T accumulate in fp32 with f32r / fp16
operands; P is cast to bf16 by the exp activation; PV runs bf16 with
fp32 PSUM accumulation.  Measured rel err vs the fp64 oracle: 1.54e-2
(threshold 2e-2) at ~565us, 84% MFU.

Startup DMAs are split by first-consumer order (fp16 rt in i-pair
slabs, fp16 et in halves, X^T_own halves last) across the two HWDGE
queues so the first W matmul issues at ~14us; xb blocks stream on the
gpsimd software-DGE queue.
"""
import numpy as np

import concourse.bass as bass_mod
import concourse.bacc as bacc
import concourse.mybir as mybir
from concourse import tile
from concourse.bass_utils import run_bass_kernel_spmd
from concourse.masks import make_identity

DT = mybir.dt
F32 = DT.float32
F32R = DT.float32r
BF16 = DT.bfloat16
F16 = DT.float16
ALU = mybir.AluOpType
ACTF = mybir.ActivationFunctionType

S, D, NCORES = 8192, 1024, 8
SL = S // NCORES          # 1024 queries per core
P = 128                   # partitions
DC = D // P               # 8 contraction chunks
MC = SL // P              # 8 query chunks per core
CB = 512                  # key block size
NV = S // CB              # 16 key blocks
TC = CB // P              # 4 t-chunks per block
SCALE = 1.0 / 32.0        # 1/sqrt(D)
NEG_BIG = -1.0e30


def build_program(num_devices=NCORES):
    nc = bacc.Bacc("TRN2", target_bir_lowering=False, debug=False,
                   num_devices=num_devices)

    rt_p = nc.declare_dram_parameter("rt", [D, D], F16, isOutput=False)
    et_p = nc.declare_dram_parameter("et", [D, D], F16, isOutput=False)
    # X^T pre-blocked by key block: [NV, D, CB] flattened to [NV*D, CB]
    xtb_p = nc.declare_dram_parameter("xtb", [NV * D, CB], F16,
                                      isOutput=False)
    # own X^T slice in f32r for the T^T projection (Q side stays clean)
    xt_p = nc.declare_dram_parameter("xt", [D, SL], F32R, isOutput=False)
    xbf_p = nc.declare_dram_parameter("xbf", [S, D], BF16, isOutput=False)
    out_p = nc.declare_dram_parameter("out", [SL, D], F32, isOutput=True)

    with tile.TileContext(nc) as tc:
        with (
            tc.tile_pool(name="persist", bufs=1) as pers,
        ):
            tq = pers.tile([P, DC * SL], F16, tag="tq")       # T^T, [c|m]
            oacc = pers.tile([P, MC * D], F32, tag="oacc")    # O accum per m
            ident_bf = pers.tile([P, P], BF16, tag="identbf")
            mst = [[pers.tile([P, 1], F32, tag=f"mst{m}_{j}",
                              name=f"mst{m}_{j}")
                    for j in range(2)] for m in range(MC)]
            sig = [pers.tile([P, 1], F32, tag=f"sig{m}", name=f"sig{m}")
                   for m in range(MC)]

            with tc.tile_pool(name="ident_tmp", bufs=1) as identp:
                ident32 = identp.tile([P, P], F32, tag="ident32")
                make_identity(nc, ident32[:])
                nc.vector.tensor_copy(ident_bf[:], ident32[:])
            nc.vector.memset(oacc[:], 0.0)
            for m in range(MC):
                nc.vector.memset(mst[m][0][:], NEG_BIG)
                nc.vector.memset(sig[m][:], 0.0)

            pid_sy = nc.sync.partition_id()
            pid_sc = nc.scalar.partition_id()
            pid_gp = nc.gpsimd.partition_id()

            with (
                tc.tile_pool(name="kt", bufs=2) as ktp,
                tc.tile_pool(name="xb", bufs=2) as xbp,
            ):
                def load_kt(j, eng, pid):
                    kt = ktp.tile([P, DC * CB], F16, tag="kt", name="kt")
                    eng.dma_start(
                        kt.rearrange("p (k c) -> p k c", k=DC),
                        xtb_p[bass_mod.ds(
                            ((pid * 2 + j) % NV) * D, D), :]
                        .rearrange("(k p) c -> p k c", p=P))
                    return kt

                def load_xb(j):
                    xb = xbp.tile([P, TC * D], BF16, tag="xb", name="xb")
                    nc.gpsimd.dma_start(
                        xb.rearrange("p (k c) -> p k c", k=TC),
                        xbf_p[bass_mod.ds(
                            ((pid_gp * 2 + j) % NV) * CB, CB), :]
                        .rearrange("(k p) c -> p k c", p=P))
                    return xb

                # own key blocks double as X^T_own for the T^T projection
                xb0 = load_xb(0)
                xb1 = load_xb(1)

                # ------------ Phase A: W = R @ E^T, then T^T ------------
                with (
                    tc.tile_pool(name="pa", bufs=1) as pa,
                    tc.tile_pool(name="rt", bufs=2) as rtp,
                    tc.tile_pool(name="pa_ps", bufs=2, space="PSUM") as pa_ps,
                ):
                    et_sb = pa.tile([P, DC * D], F16, tag="et")   # [d | j]
                    w_sb = pa.tile([P, DC * D], F32R, tag="w")    # [i | j]
                    xt_sb = pa.tile([P, DC * SL], F32R, tag="xt")  # [d | m]
                    et_r = et_sb.rearrange("p (k c) -> p k c", k=DC)
                    et_src = et_p.rearrange("(k p) c -> p k c", p=P)
                    for jh in range(2):
                        nc.scalar.dma_start(
                            et_r[:, :, jh * CB:(jh + 1) * CB],
                            et_src[:, :, jh * CB:(jh + 1) * CB])
                    xt_r = xt_sb.rearrange("p (k c) -> p k c", k=DC)
                    xt_src = xt_p.rearrange("(k p) c -> p k c", p=P)
                    rt_src = rt_p.rearrange("(k p) c -> p k c", p=P)

                    def load_rt(g):
                        # 1MB fp16 slab of 4 i-blocks (1KB DMA lines)
                        rt_t = rtp.tile([P, DC * 4 * P], F16, tag="rtg",
                                        name="rtg")
                        nc.sync.dma_start(
                            rt_t.rearrange("p (k c) -> p k c", k=DC),
                            rt_src[:, :, g * 4 * P:(g + 1) * 4 * P])
                        return rt_t

                    # W[i, j] = sum_d R^T[d, i] * E^T[d, j]
                    rt_tiles = {0: load_rt(0), 1: load_rt(1)}
                    for g in range(2):
                        rt_t = rt_tiles.pop(g)
                        for i2 in range(4):
                            i = g * 4 + i2
                            for jh in range(2):
                                ps = pa_ps.tile([P, CB], F32, tag="proj")
                                for k in range(DC):
                                    nc.tensor.matmul(
                                        ps[:],
                                        rt_t[:, k * 4 * P + i2 * P:
                                             k * 4 * P + (i2 + 1) * P],
                                        et_sb[:, k * D + jh * CB:
                                              k * D + (jh + 1) * CB],
                                        start=(k == 0), stop=(k == DC - 1),
                                    )
                                nc.vector.tensor_copy(
                                    w_sb[:, i * D + jh * CB:
                                         i * D + (jh + 1) * CB],
                                    ps[:])
                        if g == 0:
                            # own X^T quarters interleave on both queues,
                            # mh=1 half (consumed first by T^T) leading
                            nc.sync.dma_start(
                                xt_r[:, :, CB:CB + 256],
                                xt_src[:, :, CB:CB + 256])
                            nc.scalar.dma_start(
                                xt_r[:, :, CB + 256:2 * CB],
                                xt_src[:, :, CB + 256:2 * CB])
                    nc.sync.dma_start(xt_r[:, :, 0:256],
                                      xt_src[:, :, 0:256])
                    nc.scalar.dma_start(xt_r[:, :, 256:CB],
                                        xt_src[:, :, 256:CB])
                    kt0 = load_kt(0, nc.sync, pid_sy)
                    kt1 = load_kt(1, nc.scalar, pid_sc)

                    # T^T[c, m] = sum_d W[d, c] * X^T[d, m]
                    # mh=1 first: its X^T half lands on the less-loaded queue
                    for mh in (1, 0):
                        for c in range(DC):
                            ps = pa_ps.tile([P, CB], F32, tag="proj")
                            for k in range(DC):
                                nc.tensor.matmul(
                                    ps[:],
                                    w_sb[:, k * D + c * P:
                                         k * D + (c + 1) * P],
                                    xt_sb[:, k * SL + mh * CB:
                                          k * SL + (mh + 1) * CB],
                                    start=(k == 0), stop=(k == DC - 1),
                                )
                            nc.vector.tensor_copy(
                                tq[:, c * SL + mh * CB:
                                   c * SL + (mh + 1) * CB],
                                ps[:])

                # ------------- Phase B: blocked attention ---------------
                # 16 key blocks of 512 in ring order starting at the own
                # blocks.  Software-pipelined: PE runs PV of a previous
                # block's m while DVE/ACT compute stats+exp of the current.
                self_attention_pools = (
                    tc.tile_pool(name="ph", bufs=4),
                    tc.tile_pool(name="pt", bufs=3),
                    tc.tile_pool(name="of", bufs=2),
                    tc.tile_pool(name="stats", bufs=6),
                    tc.tile_pool(name="s_ps", bufs=3, space="PSUM"),
                    tc.tile_pool(name="t_ps", bufs=2, space="PSUM"),
                    tc.tile_pool(name="o_ps", bufs=3, space="PSUM"),
                )
                with (
                    self_attention_pools[0] as php,
                    self_attention_pools[1] as ptp,
                    self_attention_pools[2] as ofp,
                    self_attention_pools[3] as stp,
                    self_attention_pools[4] as sps,
                    self_attention_pools[5] as tps,
                    self_attention_pools[6] as ops,
                ):
                    NH = D // CB  # PV output halves (separate PSUM banks)
                    def flush_dve(pend, o_halves):
                        ph, alpha, m, j, xb, pt = pend
                        for h, o_h in enumerate(o_halves):
                            nc.vector.scalar_tensor_tensor(
                                oacc[:, m * D + h * CB:
                                     m * D + (h + 1) * CB],
                                oacc[:, m * D + h * CB:
                                     m * D + (h + 1) * CB],
                                alpha[:], o_h[:],
                                op0=ALU.mult, op1=ALU.add)
                        if j == NV - 1:
                            # finalize this m: divide by softmax sum, store
                            rcp = stp.tile([P, 1], F32, tag="rcp",
                                           name="rcp")
                            nc.vector.reciprocal(rcp[:], sig[m][:])
                            of = ofp.tile([P, D], F32, tag="ofin",
                                          name="ofin")
                            for h, eng in ((0, nc.sync), (1, nc.scalar)):
                                nc.vector.tensor_scalar_mul(
                                    of[:, h * CB:(h + 1) * CB],
                                    oacc[:, m * D + h * CB:
                                         m * D + (h + 1) * CB], rcp[:])
                                eng.dma_start(
                                    out_p[m * P:(m + 1) * P,
                                          h * CB:(h + 1) * CB],
                                    of[:, h * CB:(h + 1) * CB])

                    def emit_step(tr, pv):
                        # Interleave the LDW-bound PE transposes of pending
                        # `tr` between the PV matmuls of pending `pv` so the
                        # transpose weight loads hide under the 213ns PV MMs.
                        tp = None
                        if tr is not None:
                            tp = tps.tile([P, CB], BF16, tag="tp", name="tp")
                        o_halves = None
                        if pv is not None:
                            o_halves = [ops.tile([P, CB], F32, tag="opart",
                                                 name="o_part")
                                        for _ in range(NH)]
                        for tc_ in range(TC):
                            if tr is not None:
                                nc.tensor.transpose(
                                    tp[:, tc_ * P:(tc_ + 1) * P],
                                    tr[0][:, tc_ * P:(tc_ + 1) * P],
                                    ident_bf[:],
                                )
                            if pv is not None:
                                pt, xb = pv[5], pv[4]
                                for h in range(NH):
                                    nc.tensor.matmul(
                                        o_halves[h][:],
                                        pt[:, tc_ * P:(tc_ + 1) * P],
                                        xb[:, tc_ * D + h * CB:
                                           tc_ * D + (h + 1) * CB],
                                        start=(tc_ == 0),
                                        stop=(tc_ == TC - 1),
                                    )
                        if tr is not None:
                            pt_new = ptp.tile([P, CB], BF16, tag="pt",
                                              name="pt")
                            nc.scalar.copy(pt_new[:], tp[:])
                            tr[5] = pt_new
                        if pv is not None:
                            flush_dve(pv, o_halves)

                    pend_s = []   # stats done, needs transpose
                    pend_t = []   # transposed, needs PV
                    for j in range(NV):
                        if j == 0:
                            kt, xb = kt0, xb0
                        elif j == 1:
                            kt, xb = kt1, xb1
                        else:
                            kt = load_kt(j, nc.sync if j % 2 == 0
                                         else nc.scalar,
                                         pid_sy if j % 2 == 0 else pid_sc)
                            xb = load_xb(j)

                        for m in range(MC):
                            sh = sps.tile([P, CB], F32, tag="s", name="s")
                            for k in range(DC):
                                nc.tensor.matmul(
                                    sh[:],
                                    tq[:, k * SL + m * P:
                                       k * SL + (m + 1) * P],
                                    kt[:, k * CB:(k + 1) * CB],
                                    start=(k == 0), stop=(k == DC - 1),
                                )
                            mq = stp.tile([P, 1], F32, tag="mq", name="mq")
                            nc.vector.reduce_max(mq[:], sh[:],
                                                 axis=mybir.AxisListType.X)

                            # online softmax stats; mst ping-pongs on j
                            m_old = mst[m][j % 2]
                            mnew = mst[m][(j + 1) % 2]
                            nc.vector.tensor_max(mnew[:], m_old[:], mq[:])
                            nbias = stp.tile([P, 1], F32, tag="nbias",
                                             name="nbias")
                            nc.scalar.mul(nbias[:], mnew[:], -SCALE)
                            # alpha = exp((m_old - mnew)/32)
                            alpha = stp.tile([P, 1], F32, tag="alpha",
                                             name="alpha")
                            nc.scalar.activation(alpha[:], m_old[:],
                                                 ACTF.Exp,
                                                 bias=nbias[:], scale=SCALE)

                            # phat = exp(s/32 - mnew/32) in bf16; sums in sq
                            sq = stp.tile([P, 1], F32, tag="sq", name="sq")
                            ph = php.tile([P, CB], BF16, tag="ph",
                                          name="ph")
                            nc.scalar.activation(ph[:], sh[:], ACTF.Exp,
                                                 bias=nbias[:], scale=SCALE,
                                                 accum_out=sq[:])
                            nc.vector.scalar_tensor_tensor(
                                sig[m][:], sig[m][:], alpha[:], sq[:],
                                op0=ALU.mult, op1=ALU.add)

                            pend_s.append([ph, alpha, m, j, xb, None])
                            if len(pend_s) >= 2:
                                tr = pend_s.pop(0)
                                pv = pend_t.pop(0) if pend_t else None
                                emit_step(tr, pv)
                                pend_t.append(tr)
                    while pend_s or pend_t:
                        tr = pend_s.pop(0) if pend_s else None
                        pv = pend_t.pop(0) if pend_t else None
                        emit_step(tr, pv)
                        if tr is not None:
                            pend_t.append(tr)

    nc.compile()
    return nc


_PROGRAM = None


def _get_program():
    global _PROGRAM
    if _PROGRAM is None:
        _PROGRAM = build_program()
    return _PROGRAM


def kernel(inputs, rotation_params, entangle_params, _trace=False):
    X = np.ascontiguousarray(np.asarray(inputs, dtype=np.float32))
    R = np.ascontiguousarray(np.asarray(rotation_params, dtype=np.float32))
    E = np.ascontiguousarray(np.asarray(entangle_params, dtype=np.float32))
    assert X.shape == (S, D) and R.shape == (D, D) and E.shape == (D, D)

    import ml_dtypes
    XT = np.ascontiguousarray(X.T)
    RT = np.ascontiguousarray(R.T.astype(np.float16))
    ET = np.ascontiguousarray(E.T.astype(np.float16))
    # X^T pre-blocked by key block in fp16: [NV, D, CB] -> [NV*D, CB]
    XTB = np.ascontiguousarray(
        XT.astype(np.float16).reshape(D, NV, CB)
        .transpose(1, 0, 2)).reshape(NV * D, CB)
    Xbf = np.ascontiguousarray(X.astype(ml_dtypes.bfloat16))
    in_maps = []
    for i in range(NCORES):
        in_maps.append({
            "rt": RT,
            "et": ET,
            "xtb": XTB,
            "xt": np.ascontiguousarray(XT[:, i * SL:(i + 1) * SL]),
            "xbf": Xbf,
        })

    nc = _get_program()
    res = run_bass_kernel_spmd(nc, in_maps, list(range(NCORES)),
                               trace=_trace)
    out = np.concatenate([res.results[i]["out"] for i in range(NCORES)],
                         axis=0)
    if _trace:
        return out, res
    return out


# revision 30
# speedup vs baseline: 1.0017x; 1.0017x over previous
"""Trainium2 Bass kernel for ClassicalSelfAttention.

  out = softmax((X @ R) @ (X @ E).T / sqrt(D)) @ X,  X: (8192, 1024) fp32

Key identity: scores = (X R)(X E)^T = X (R E^T) X^T.  Each core computes
W = R @ E^T redundantly (27us of PE), projects its own query slice
(T^T with lhsT=W, rhs=X^T_own), and then the "keys" are just X^T itself
— which every core already holds in DRAM.  No collectives at all: the
attention loop streams X^T / X blocks straight from HBM.

Sequence-parallel over 8 NeuronCores: core i owns queries
[i*1024, (i+1)*1024).  Attention runs over 16 key blocks of 512 in ring
order starting at the core's own two blocks, with a standard online
softmax (per-query running max on DVE, exp on ACT, PV accumulated in
per-half PSUM banks then merged into SBUF by DVE).  P^T for the PV
matmul is produced by PE transposes against a bf16 identity,
interleaved between the previous pending's PV matmuls so the
LDW-bound transposes hide under 213ns PV streams.

Precision: the softmax is extremely peaked (logit std ~1200 scaled;
argmax gaps down to ~3), so logits need >=11-bit operand mantissas —
bf16 flips argmaxes and fails.  QK runs in fp16 x fp16 (exact MACs,
one-pass LDWEIGHTS); W and T^T accumulate in fp32 with f32r / fp16
operands; P is cast to bf16 by the exp activation; PV runs bf16 with
fp32 PSUM accumulation.  Measured rel err vs the fp64 oracle: 1.54e-2
(threshold 2e-2) at ~565us, 84% MFU.

Startup DMAs are split by first-consumer order (fp16 rt in i-pair
slabs, fp16 et in halves, X^T_own halves last) across the two HWDGE
queues so the first W matmul issues at ~14us; xb blocks stream on the
gpsimd software-DGE queue.
"""
import numpy as np

import concourse.bass as bass_mod
import concourse.bacc as bacc
import concourse.mybir as mybir
from concourse import tile
from concourse.bass_utils import run_bass_kernel_spmd
from concourse.masks import make_identity

DT = mybir.dt
F32 = DT.float32
F32R = DT.float32r
BF16 = DT.bfloat16
F16 = DT.float16
ALU = mybir.AluOpType
ACTF = mybir.ActivationFunctionType

S, D, NCORES = 8192, 1024, 8
SL = S // NCORES          # 1024 queries per core
P = 128                   # partitions
DC = D // P               # 8 contraction chunks
MC = SL // P              # 8 query chunks per core
CB = 512                  # key block size
NV = S // CB              # 16 key blocks
TC = CB // P              # 4 t-chunks per block
SCALE = 1.0 / 32.0        # 1/sqrt(D)
NEG_BIG = -1.0e30


def build_program(num_devices=NCORES):
    nc = bacc.Bacc("TRN2", target_bir_lowering=False, debug=False,
                   num_devices=num_devices)

    rt_p = nc.declare_dram_parameter("rt", [D, D], F16, isOutput=False)
    et_p = nc.declare_dram_parameter("et", [D, D], F16, isOutput=False)
    # X^T pre-blocked by key block: [NV, D, CB] flattened to [NV*D, CB]
    xtb_p = nc.declare_dram_parameter("xtb", [NV * D, CB], F16,
                                      isOutput=False)
    # own X^T slice in f32r for the T^T projection (Q side stays clean)
    xt_p = nc.declare_dram_parameter("xt", [D, SL], F32R, isOutput=False)
    xbf_p = nc.declare_dram_parameter("xbf", [S, D], BF16, isOutput=False)
    out_p = nc.declare_dram_parameter("out", [SL, D], F32, isOutput=True)

    with tile.TileContext(nc) as tc:
        with (
            tc.tile_pool(name="persist", bufs=1) as pers,
        ):
            tq = pers.tile([P, DC * SL], F16, tag="tq")       # T^T, [c|m]
            oacc = pers.tile([P, MC * D], F32, tag="oacc")    # O accum per m
            ident_bf = pers.tile([P, P], BF16, tag="identbf")
            mst = [[pers.tile([P, 1], F32, tag=f"mst{m}_{j}",
                              name=f"mst{m}_{j}")
                    for j in range(2)] for m in range(MC)]
            sig = [pers.tile([P, 1], F32, tag=f"sig{m}", name=f"sig{m}")
                   for m in range(MC)]

            with tc.tile_pool(name="ident_tmp", bufs=1) as identp:
                ident32 = identp.tile([P, P], F32, tag="ident32")
                make_identity(nc, ident32[:])
                nc.vector.tensor_copy(ident_bf[:], ident32[:])
            nc.vector.memset(oacc[:], 0.0)
            for m in range(MC):
                nc.vector.memset(mst[m][0][:], NEG_BIG)
                nc.vector.memset(sig[m][:], 0.0)

            pid_sy = nc.sync.partition_id()
            pid_sc = nc.scalar.partition_id()
            pid_gp = nc.gpsimd.partition_id()

            with (
                tc.tile_pool(name="kt", bufs=2) as ktp,
                tc.tile_pool(name="xb", bufs=2) as xbp,
            ):
                def load_kt(j, eng, pid):
                    kt = ktp.tile([P, DC * CB], F16, tag="kt", name="kt")
                    eng.dma_start(
                        kt.rearrange("p (k c) -> p k c", k=DC),
                        xtb_p[bass_mod.ds(
                            ((pid * 2 + j) % NV) * D, D), :]
                        .rearrange("(k p) c -> p k c", p=P))
                    return kt

                def load_xb(j):
                    xb = xbp.tile([P, TC * D], BF16, tag="xb", name="xb")
                    nc.gpsimd.dma_start(
                        xb.rearrange("p (k c) -> p k c", k=TC),
                        xbf_p[bass_mod.ds(
                            ((pid_gp * 2 + j) % NV) * CB, CB), :]
                        .rearrange("(k p) c -> p k c", p=P))
                    return xb

                # own key blocks double as X^T_own for the T^T projection
                xb0 = load_xb(0)
                xb1 = load_xb(1)

                # ------------ Phase A: W = R @ E^T, then T^T ------------
                with (
                    tc.tile_pool(name="pa", bufs=1) as pa,
                    tc.tile_pool(name="rt", bufs=2) as rtp,
                    tc.tile_pool(name="pa_ps", bufs=2, space="PSUM") as pa_ps,
                ):
                    et_sb = pa.tile([P, DC * D], F16, tag="et")   # [d | j]
                    w_sb = pa.tile([P, DC * D], F32R, tag="w")    # [i | j]
                    xt_sb = pa.tile([P, DC * SL], F32R, tag="xt")  # [d | m]
                    et_r = et_sb.rearrange("p (k c) -> p k c", k=DC)
                    et_src = et_p.rearrange("(k p) c -> p k c", p=P)
                    for jh in range(2):
                        nc.scalar.dma_start(
                            et_r[:, :, jh * CB:(jh + 1) * CB],
                            et_src[:, :, jh * CB:(jh + 1) * CB])
                    xt_r = xt_sb.rearrange("p (k c) -> p k c", k=DC)
                    xt_src = xt_p.rearrange("(k p) c -> p k c", p=P)
                    rt_src = rt_p.rearrange("(k p) c -> p k c", p=P)

                    def load_rt(g):
                        # 0.5MB fp16 slab of 2 i-blocks
                        rt_t = rtp.tile([P, DC * 2 * P], F16, tag="rtg",
                                        name="rtg")
                        nc.sync.dma_start(
                            rt_t.rearrange("p (k c) -> p k c", k=DC),
                            rt_src[:, :, g * 2 * P:(g + 1) * 2 * P])
                        return rt_t

                    # W[i, j] = sum_d R^T[d, i] * E^T[d, j]
                    NG = DC // 2
                    rt_tiles = {0: load_rt(0), 1: load_rt(1)}
                    for g in range(NG):
                        rt_t = rt_tiles.pop(g)
                        for i2 in range(2):
                            i = g * 2 + i2
                            for jh in range(2):
                                ps = pa_ps.tile([P, CB], F32, tag="proj")
                                for k in range(DC):
                                    nc.tensor.matmul(
                                        ps[:],
                                        rt_t[:, k * 2 * P + i2 * P:
                                             k * 2 * P + (i2 + 1) * P],
                                        et_sb[:, k * D + jh * CB:
                                              k * D + (jh + 1) * CB],
                                        start=(k == 0), stop=(k == DC - 1),
                                    )
                                nc.vector.tensor_copy(
                                    w_sb[:, i * D + jh * CB:
                                         i * D + (jh + 1) * CB],
                                    ps[:])
                        if g + 2 < NG:
                            rt_tiles[g + 2] = load_rt(g + 2)
                        elif g + 2 == NG:
                            # own X^T quarters interleave on both queues,
                            # mh=1 half (consumed first by T^T) leading
                            nc.sync.dma_start(
                                xt_r[:, :, CB:CB + 256],
                                xt_src[:, :, CB:CB + 256])
                            nc.scalar.dma_start(
                                xt_r[:, :, CB + 256:2 * CB],
                                xt_src[:, :, CB + 256:2 * CB])
                    nc.sync.dma_start(xt_r[:, :, 0:256],
                                      xt_src[:, :, 0:256])
                    nc.scalar.dma_start(xt_r[:, :, 256:CB],
                                        xt_src[:, :, 256:CB])
                    kt0 = load_kt(0, nc.sync, pid_sy)
                    kt1 = load_kt(1, nc.scalar, pid_sc)

                    # T^T[c, m] = sum_d W[d, c] * X^T[d, m]
                    # mh=1 first: its X^T half lands on the less-loaded queue
                    for mh in (1, 0):
                        for c in range(DC):
                            ps = pa_ps.tile([P, CB], F32, tag="proj")
                            for k in range(DC):
                                nc.tensor.matmul(
                                    ps[:],
                                    w_sb[:, k * D + c * P:
                                         k * D + (c + 1) * P],
                                    xt_sb[:, k * SL + mh * CB:
                                          k * SL + (mh + 1) * CB],
                                    start=(k == 0), stop=(k == DC - 1),
                                )
                            nc.vector.tensor_copy(
                                tq[:, c * SL + mh * CB:
                                   c * SL + (mh + 1) * CB],
                                ps[:])

                # ------------- Phase B: blocked attention ---------------
                # 16 key blocks of 512 in ring order starting at the own
                # blocks.  Software-pipelined: PE runs PV of a previous
                # block's m while DVE/ACT compute stats+exp of the current.
                self_attention_pools = (
                    tc.tile_pool(name="ph", bufs=4),
                    tc.tile_pool(name="pt", bufs=3),
                    tc.tile_pool(name="of", bufs=2),
                    tc.tile_pool(name="stats", bufs=6),
                    tc.tile_pool(name="s_ps", bufs=3, space="PSUM"),
                    tc.tile_pool(name="t_ps", bufs=2, space="PSUM"),
                    tc.tile_pool(name="o_ps", bufs=3, space="PSUM"),
                )
                with (
                    self_attention_pools[0] as php,
                    self_attention_pools[1] as ptp,
                    self_attention_pools[2] as ofp,
                    self_attention_pools[3] as stp,
                    self_attention_pools[4] as sps,
                    self_attention_pools[5] as tps,
                    self_attention_pools[6] as ops,
                ):
                    NH = D // CB  # PV output halves (separate PSUM banks)
                    def flush_dve(pend, o_halves):
                        ph, alpha, m, j, xb, pt = pend
                        for h, o_h in enumerate(o_halves):
                            nc.vector.scalar_tensor_tensor(
                                oacc[:, m * D + h * CB:
                                     m * D + (h + 1) * CB],
                                oacc[:, m * D + h * CB:
                                     m * D + (h + 1) * CB],
                                alpha[:], o_h[:],
                                op0=ALU.mult, op1=ALU.add)
                        if j == NV - 1:
                            # finalize this m: divide by softmax sum, store
                            rcp = stp.tile([P, 1], F32, tag="rcp",
                                           name="rcp")
                            nc.vector.reciprocal(rcp[:], sig[m][:])
                            of = ofp.tile([P, D], F32, tag="ofin",
                                          name="ofin")
                            for h, eng in ((0, nc.sync), (1, nc.scalar)):
                                nc.vector.tensor_scalar_mul(
                                    of[:, h * CB:(h + 1) * CB],
                                    oacc[:, m * D + h * CB:
                                         m * D + (h + 1) * CB], rcp[:])
                                eng.dma_start(
                                    out_p[m * P:(m + 1) * P,
                                          h * CB:(h + 1) * CB],
                                    of[:, h * CB:(h + 1) * CB])

                    def emit_step(tr, pv):
                        # Interleave the LDW-bound PE transposes of pending
                        # `tr` between the PV matmuls of pending `pv` so the
                        # transpose weight loads hide under the 213ns PV MMs.
                        tp = None
                        if tr is not None:
                            tp = tps.tile([P, CB], BF16, tag="tp", name="tp")
                        o_halves = None
                        if pv is not None:
                            o_halves = [ops.tile([P, CB], F32, tag="opart",
                                                 name="o_part")
                                        for _ in range(NH)]
                        for tc_ in range(TC):
                            if tr is not None:
                                nc.tensor.transpose(
                                    tp[:, tc_ * P:(tc_ + 1) * P],
                                    tr[0][:, tc_ * P:(tc_ + 1) * P],
                                    ident_bf[:],
                                )
                            if pv is not None:
                                pt, xb = pv[5], pv[4]
                                for h in range(NH):
                                    nc.tensor.matmul(
                                        o_halves[h][:],
                                        pt[:, tc_ * P:(tc_ + 1) * P],
                                        xb[:, tc_ * D + h * CB:
                                           tc_ * D + (h + 1) * CB],
                                        start=(tc_ == 0),
                                        stop=(tc_ == TC - 1),
                                    )
                        if tr is not None:
                            pt_new = ptp.tile([P, CB], BF16, tag="pt",
                                              name="pt")
                            nc.scalar.copy(pt_new[:], tp[:])
                            tr[5] = pt_new
                        if pv is not None:
                            flush_dve(pv, o_halves)

                    pend_s = []   # stats done, needs transpose
                    pend_t = []   # transposed, needs PV
                    for j in range(NV):
                        if j == 0:
                            kt, xb = kt0, xb0
                        elif j == 1:
                            kt, xb = kt1, xb1
                        else:
                            kt = load_kt(j, nc.sync if j % 2 == 0
                                         else nc.scalar,
                                         pid_sy if j % 2 == 0 else pid_sc)
                            xb = load_xb(j)

                        for m in range(MC):
                            sh = sps.tile([P, CB], F32, tag="s", name="s")
                            for k in range(DC):
                                nc.tensor.matmul(
                                    sh[:],
                                    tq[:, k * SL + m * P:
                                       k * SL + (m + 1) * P],
                                    kt[:, k * CB:(k + 1) * CB],
                                    start=(k == 0), stop=(k == DC - 1),
                                )
                            mq = stp.tile([P, 1], F32, tag="mq", name="mq")
                            nc.vector.reduce_max(mq[:], sh[:],
                                                 axis=mybir.AxisListType.X)

                            # online softmax stats; mst ping-pongs on j
                            m_old = mst[m][j % 2]
                            mnew = mst[m][(j + 1) % 2]
                            nc.vector.tensor_max(mnew[:], m_old[:], mq[:])
                            nbias = stp.tile([P, 1], F32, tag="nbias",
                                             name="nbias")
                            nc.scalar.mul(nbias[:], mnew[:], -SCALE)
                            # alpha = exp((m_old - mnew)/32)
                            alpha = stp.tile([P, 1], F32, tag="alpha",
                                             name="alpha")
                            nc.scalar.activation(alpha[:], m_old[:],
                                                 ACTF.Exp,
                                                 bias=nbias[:], scale=SCALE)

                            # phat = exp(s/32 - mnew/32) in bf16; sums in sq
                            sq = stp.tile([P, 1], F32, tag="sq", name="sq")
                            ph = php.tile([P, CB], BF16, tag="ph",
                                          name="ph")
                            nc.scalar.activation(ph[:], sh[:], ACTF.Exp,
                                                 bias=nbias[:], scale=SCALE,
                                                 accum_out=sq[:])
                            nc.vector.scalar_tensor_tensor(
                                sig[m][:], sig[m][:], alpha[:], sq[:],
                                op0=ALU.mult, op1=ALU.add)

                            pend_s.append([ph, alpha, m, j, xb, None])
                            if len(pend_s) >= 2:
                                tr = pend_s.pop(0)
                                pv = pend_t.pop(0) if pend_t else None
                                emit_step(tr, pv)
                                pend_t.append(tr)
                    while pend_s or pend_t:
                        tr = pend_s.pop(0) if pend_s else None
                        pv = pend_t.pop(0) if pend_t else None
                        emit_step(tr, pv)
                        if tr is not None:
                            pend_t.append(tr)

    nc.compile()
    return nc


_PROGRAM = None


def _get_program():
    global _PROGRAM
    if _PROGRAM is None:
        _PROGRAM = build_program()
    return _PROGRAM


def kernel(inputs, rotation_params, entangle_params, _trace=False):
    X = np.ascontiguousarray(np.asarray(inputs, dtype=np.float32))
    R = np.ascontiguousarray(np.asarray(rotation_params, dtype=np.float32))
    E = np.ascontiguousarray(np.asarray(entangle_params, dtype=np.float32))
    assert X.shape == (S, D) and R.shape == (D, D) and E.shape == (D, D)

    import ml_dtypes
    XT = np.ascontiguousarray(X.T)
    RT = np.ascontiguousarray(R.T.astype(np.float16))
    ET = np.ascontiguousarray(E.T.astype(np.float16))
    # X^T pre-blocked by key block in fp16: [NV, D, CB] -> [NV*D, CB]
    XTB = np.ascontiguousarray(
        XT.astype(np.float16).reshape(D, NV, CB)
        .transpose(1, 0, 2)).reshape(NV * D, CB)
    Xbf = np.ascontiguousarray(X.astype(ml_dtypes.bfloat16))
    in_maps = []
    for i in range(NCORES):
        in_maps.append({
            "rt": RT,
            "et": ET,
            "xtb": XTB,
            "xt": np.ascontiguousarray(XT[:, i * SL:(i + 1) * SL]),
            "xbf": Xbf,
        })

    nc = _get_program()
    res = run_bass_kernel_spmd(nc, in_maps, list(range(NCORES)),
                               trace=_trace)
    out = np.concatenate([res.results[i]["out"] for i in range(NCORES)],
                         axis=0)
    if _trace:
        return out, res
    return out
